# revision 14
# baseline (speedup 1.0000x reference)
"""Trainium2 Bass kernel for nn_AttentionWavelet (bior4.4 3-level DWT + channel
softmax attention + per-channel grouped 1x1-conv MLP encoders).

Self-contained: hardcodes shapes/sharding. Takes FULL inputs, returns FULL output.

Sharding: pure data parallel over 8 cores = (batch b 0..3) x (image H-half 0..1),
with a 64-row halo so each core computes its half's DWT independently.

Per-core device pipeline (one SPMD Bass program, per-core data):
  L1/L2/L3 DWT as banded matmuls on PE (contract partition axis) + PE transposes,
  channel-softmax attention on DVE/ACT, and the 4-layer per-channel MLPs as
  block-diagonal matmuls (4 channels packed per 128-partition tile, f32r) with
  tanh on the scalar engine. Outputs stream straight to DRAM.

Built on bacc.Bacc (whose compile() legalizes per-instruction semaphore-wait
limits); constants arrive in two packed DMA loads.
"""
import os
import sys

import numpy as np

for _p in ('/opt/trn_rl_repo', '/root/.axon_site/_ro/trn_rl_repo'):
    if os.path.isdir(_p) and _p not in sys.path:
        sys.path.append(_p)

DEC_LO = np.array([0.0, 0.03782845550726404, -0.023849465019556843,
                   -0.11062440441843718, 0.37740285561283066, 0.8526986790088938,
                   0.37740285561283066, -0.11062440441843718,
                   -0.023849465019556843, 0.03782845550726404], dtype=np.float64)
DEC_HI = np.array([0.0, -0.06453888262869706, 0.04068941760916406,
                   0.41809227322161724, -0.7884856164055829, 0.41809227322161724,
                   0.04068941760916406, -0.06453888262869706, 0.0, 0.0], dtype=np.float64)
FILT = {'lo': DEC_LO, 'hi': DEC_HI}

# MLP matmul dtype: 'f32r' (fast, reduced precision) or 'f32'
MLP_DT = os.environ.get('AW_MLP_DT', 'f32r')

# virtual-channel order per (level, group)
GROUP_VCH = {
    1: [[0, 1, 2, 3], [4, 5, 6, 7], [8, 8, 8, 8]],
    2: [[0, 1, 2, 3], [4, 5, 6, 7], [8, 8, 8, 8]],
    3: [[0, 1, 2, 3], [4, 5, 6, 7], [8, 8, 8, 8]],
    'yl': [[0, 1, 2, None]],
}
BSEL = {'lh': (0, 1), 'hl': (1, 0), 'hh': (1, 1)}   # band -> (wb, hb)
BANDS = ('lh', 'hl', 'hh')
# output row slot -> channel (host reorders):
SLOTS = {lvl: GROUP_VCH[lvl][0] + GROUP_VCH[lvl][1] + [8] for lvl in (1, 2, 3)}
LGS = [(1, 0), (1, 1), (1, 2), (2, 0), (2, 1), (2, 2), (3, 0), (3, 1), (3, 2), ('yl', 0)]
LVL_NPX = {1: 32768, 2: 8192, 3: 2048, 'yl': 2048}


# ---------------------------------------------------------------------------
# host-side constant builders
# ---------------------------------------------------------------------------

def band_mats(Hc, in_tiles, out_specs, taps_off=-5):
    out = []
    for spec in out_specs:
        row = []
        for (start, nrows) in in_tiles:
            F = np.zeros((nrows, len(spec)), dtype=np.float64)
            for j, (fname, m) in enumerate(spec):
                f = FILT[fname]
                for l in range(10):
                    r = (2 * m + l + taps_off) % Hc - start
                    if 0 <= r < nrows:
                        F[r, j] += f[l]
            row.append(F.astype(np.float32) if np.any(F) else None)
        out.append(row)
    return out


def stack_spec(n_out_tiles, tile_m, n_lo):
    specs = []
    for t in range(n_out_tiles):
        spec = []
        for j in range(tile_m):
            R = tile_m * t + j
            spec.append(('lo', R) if R < n_lo else ('hi', R - n_lo))
        specs.append(spec)
    return specs


def make_dwt_consts():
    return {
        'F1': band_mats(384, [(0, 128), (128, 128), (256, 128)], stack_spec(3, 128, 192)),
        'G1': band_mats(512, [(0, 128), (128, 128), (256, 128), (384, 128)], stack_spec(4, 128, 256)),
        'F2': band_mats(256, [(0, 128), (128, 128)], stack_spec(2, 128, 128)),
        'G2': band_mats(160, [(0, 128), (128, 32)],
                        [[('lo', j) for j in range(80)], [('hi', j) for j in range(80)]]),
        'F3': band_mats(72, [(0, 72)], stack_spec(1, 64, 32), taps_off=-1),
        'G3': band_mats(128, [(0, 128)],
                        [[('lo', j) for j in range(64)], [('hi', j) for j in range(64)]]),
    }


def pack_group(vc, w1, b1, w2, b2, w3, b3, w4, b4):
    W1p = np.zeros((4, 128), np.float32)
    W2p = np.zeros((128, 128), np.float32)
    W3p = np.zeros((128, 128), np.float32)
    W4p = np.zeros((128, 4), np.float32)
    B1p = np.zeros((128, 1), np.float32)
    B2p = np.zeros((128, 1), np.float32)
    B3p = np.zeros((128, 1), np.float32)
    B4p = np.zeros((4, 1), np.float32)
    for b, c in enumerate(vc):
        if c is None:
            continue
        sl = slice(32 * b, 32 * b + 32)
        W1p[b, sl] = w1[c]
        W2p[sl, sl] = np.asarray(w2[c]).T
        W3p[sl, sl] = np.asarray(w3[c]).T
        W4p[sl, b] = w4[c]
        B1p[sl, 0] = b1[c]
        B2p[sl, 0] = b2[c]
        B3p[sl, 0] = b3[c]
        B4p[b, 0] = b4[c]
    return dict(W1=W1p, W2=W2p, W3=W3p, W4=W4p, B1=B1p, B2=B2p, B3=B3p, B4=B4p)


def make_mlp_consts(inputs):
    packs = {}
    for lvl in (1, 2, 3):
        i = lvl - 1
        args = (inputs['yh_w1'][i], inputs['yh_b1'][i], inputs['yh_w2'][i], inputs['yh_b2'][i],
                inputs['yh_w3'][i], inputs['yh_b3'][i], inputs['yh_w4'][i], inputs['yh_b4'][i])
        for g in range(3):
            packs[(lvl, g)] = pack_group(GROUP_VCH[lvl][g], *args)
    packs[('yl', 0)] = pack_group(GROUP_VCH['yl'][0],
                                  inputs['yl_w1'], inputs['yl_b1'], inputs['yl_w2'], inputs['yl_b2'],
                                  inputs['yl_w3'], inputs['yl_b3'], inputs['yl_w4'], inputs['yl_b4'])
    return packs


def _align(x, a=16):
    return (x + a - 1) // a * a


def const_layout(dwt):
    """Layouts of the two packed constant tensors.

    Returns (ents32, n32, entsr, nr): ents maps name -> (P, F, off)."""
    ents32 = {}
    off = 0

    def add32(name, P, F):
        nonlocal off
        ents32[name] = (P, F, off)
        off = _align(off + F)
    add32('ident', 128, 128)
    for key in ('F1', 'G1', 'F2', 'G2', 'F3', 'G3'):
        for t, row in enumerate(dwt[key]):
            for k, M in enumerate(row):
                if M is not None:
                    add32(f'{key}_{t}_{k}', M.shape[0], M.shape[1])
    for (lvl, g) in LGS:
        for p, shp in (('B1', (128, 1)), ('B2', (128, 1)), ('B3', (128, 1)), ('B4', (4, 1))):
            add32(f'{lvl}_{g}_{p}', shp[0], shp[1])
    n32 = _align(off)

    entsr = {}
    off = 0

    def addr_(name, P, F):
        nonlocal off
        entsr[name] = (P, F, off)
        off = _align(off + F)
    for (lvl, g) in LGS:
        for p, shp in (('W1', (4, 128)), ('W2', (128, 128)), ('W3', (128, 128)), ('W4', (128, 4))):
            addr_(f'{lvl}_{g}_{p}', shp[0], shp[1])
    nr = _align(off)
    return ents32, n32, entsr, nr


def pack_consts(dwt, packs):
    ents32, n32, entsr, nr = const_layout(dwt)
    c32 = np.zeros((128, n32), np.float32)
    c32r = np.zeros((128, nr), np.float32)
    P, F, off = ents32['ident']
    c32[0:128, off:off + 128] = np.eye(128, dtype=np.float32)
    for key in ('F1', 'G1', 'F2', 'G2', 'F3', 'G3'):
        for t, row in enumerate(dwt[key]):
            for k, M in enumerate(row):
                if M is not None:
                    P, F, off = ents32[f'{key}_{t}_{k}']
                    c32[0:P, off:off + F] = M
    for (lvl, g) in LGS:
        pk = packs[(lvl, g)]
        for p in ('B1', 'B2', 'B3', 'B4'):
            P, F, off = ents32[f'{lvl}_{g}_{p}']
            c32[0:P, off:off + F] = pk[p]
        for p in ('W1', 'W2', 'W3', 'W4'):
            P, F, off = entsr[f'{lvl}_{g}_{p}']
            c32r[0:P, off:off + F] = pk[p]
    return c32, c32r


def shard_input(x, core):
    b, half = core // 2, core % 2
    h0 = 256 * half
    rows = np.arange(h0 - 64, h0 + 320) % 512
    return np.ascontiguousarray(x[b][:, rows, :])  # [3, 384, 512]


# ---------------------------------------------------------------------------
# device program
# ---------------------------------------------------------------------------

def build_program(dwt):
    import concourse.mybir as mybir
    import concourse.tile as tile
    from concourse import bacc
    from contextlib import ExitStack

    dt = mybir.dt
    f32 = dt.float32
    AF = mybir.ActivationFunctionType
    mdt = dt.float32r if MLP_DT == 'f32r' else f32
    asf32 = (lambda ap: ap.bitcast(f32)) if MLP_DT == 'f32r' else (lambda ap: ap)

    ents32, n32, entsr, nr = const_layout(dwt)

    nc = bacc.Bacc()

    xh = nc.dram_tensor('xh', [3, 384, 512], f32, kind='ExternalInput')
    c32_d = nc.dram_tensor('c32', [128, n32], f32, kind='ExternalInput')
    c32r_d = nc.dram_tensor('c32r', [128, nr], mdt, kind='ExternalInput')

    oy1 = nc.dram_tensor('o_y1', [9, 32768], f32, kind='ExternalOutput')
    oy2 = nc.dram_tensor('o_y2', [9, 8192], f32, kind='ExternalOutput')
    oy3 = nc.dram_tensor('o_y3', [9, 2048], f32, kind='ExternalOutput')
    oxe = nc.dram_tensor('o_xe', [3, 2048], f32, kind='ExternalOutput')
    OY = {1: oy1, 2: oy2, 3: oy3, 'yl': oxe}

    with ExitStack() as ctx:
        tc = ctx.enter_context(tile.TileContext(nc))
        cpool = ctx.enter_context(tc.tile_pool(name='cpool', bufs=1))
        dpool = ctx.enter_context(tc.tile_pool(name='dpool', bufs=1))
        hpool = ctx.enter_context(tc.tile_pool(name='hpool', bufs=3))
        spool = ctx.enter_context(tc.tile_pool(name='spool', bufs=4))
        opool = ctx.enter_context(tc.tile_pool(name='opool', bufs=4))
        php = ctx.enter_context(tc.tile_pool(name='php', bufs=4, space='PSUM'))
        pop = ctx.enter_context(tc.tile_pool(name='pop', bufs=2, space='PSUM'))
        pdp = ctx.enter_context(tc.tile_pool(name='pdp', bufs=2, space='PSUM'))

        c32_sb = cpool.tile([128, n32], f32, name='c32_sb', tag='c32_sb')
        nc.sync.dma_start(out=c32_sb, in_=c32_d[:])
        c32r_sb = cpool.tile([128, nr], mdt, name='c32r_sb', tag='c32r_sb')
        nc.sync.dma_start(out=c32r_sb, in_=c32r_d[:])

        def C(name):
            P, F, off = ents32[name]
            return c32_sb[0:P, off:off + F]

        def CR(name):
            P, F, off = entsr[name]
            return c32r_sb[0:P, off:off + F]

        id_sb = C('ident')

        # single-DMA input load: xtall [128, (k, c, w)]
        xtall = dpool.tile([128, 3 * 1536], f32, name='xtall', tag='xtall')
        for c in range(3):
            nc.sync.dma_start(
                out=xtall.rearrange('p (k c w) -> p k c w', k=3, c=3)[:, :, c, :],
                in_=xh[c].rearrange('(k p) w -> p k w', p=128))
        xt = [xtall[:, 1536 * k: 1536 * k + 1536] for k in range(3)]

        def mm_stack(Fkey, in_tiles, out_free, chunks, psum_shape, out_name, out_dtype=f32):
            outs = []
            for t in range(len(dwt[Fkey])):
                ot = dpool.tile([psum_shape[0], out_free], out_dtype, name=f'{out_name}{t}', tag=f'{out_name}{t}')
                for (lo, hi) in chunks:
                    ps = pdp.tile([psum_shape[0], hi - lo], f32, name='dps', tag='dps')
                    mms = [(k, f'{Fkey}_{t}_{k}') for k in range(len(in_tiles))
                           if dwt[Fkey][t][k] is not None]
                    for i, (k, cn) in enumerate(mms):
                        nc.tensor.matmul(ps, C(cn), in_tiles[k][:, lo:hi],
                                         start=(i == 0), stop=(i == len(mms) - 1))
                    nc.vector.tensor_copy(ot[:, lo:hi], ps)
                outs.append(ot)
            return outs

        # ================= L1 =================
        s1 = mm_stack('F1', xt, 1536, [(0, 512), (512, 1024), (1024, 1536)], (128, 512), 's1_')
        s1T = [dpool.tile([128, 3 * 384], f32, name=f's1T{v}', tag=f's1T{v}') for v in range(4)]
        for t in range(3):
            for c in range(3):
                for v in range(4):
                    psT = pdp.tile([128, 128], f32, name='dpsT', tag='dps')
                    nc.tensor.transpose(psT, s1[t][:, c * 512 + 128 * v: c * 512 + 128 * v + 128], id_sb)
                    nc.vector.tensor_copy(s1T[v][:, c * 384 + 128 * t: c * 384 + 128 * t + 128], psT)
        b1 = mm_stack('G1', s1T, 1152, [(0, 384), (384, 768), (768, 1152)], (128, 384), 'b1_',
                      out_dtype=mdt)

        # ================= L2 =================
        in2 = [asf32(b1[k].rearrange('p (c m) -> p c m', c=3)[:, :, 16:176]) for k in range(2)]
        s2 = []
        for t in range(2):
            ot = dpool.tile([128, 480], f32, name=f's2_{t}', tag=f's2_{t}')
            ps = pdp.tile([128, 480], f32, name='dps2', tag='dps')
            mms = [k for k in range(2) if dwt['F2'][t][k] is not None]
            for i, k in enumerate(mms):
                nc.tensor.matmul(ps, C(f'F2_{t}_{k}'), in2[k],
                                 start=(i == 0), stop=(i == len(mms) - 1))
            nc.vector.tensor_copy(ot, ps)
            s2.append(ot)
        s2T0 = dpool.tile([128, 3 * 256], f32, name='s2T0', tag='s2T0')
        s2T1 = dpool.tile([32, 3 * 256], f32, name='s2T1', tag='s2T1')
        for t in range(2):
            for c in range(3):
                psT = pdp.tile([128, 128], f32, name='dpsT2a', tag='dps')
                nc.tensor.transpose(psT, s2[t][:, c * 160: c * 160 + 128], id_sb)
                nc.vector.tensor_copy(s2T0[:, c * 256 + 128 * t: c * 256 + 128 * t + 128], psT)
                psT2 = pdp.tile([32, 128], f32, name='dpsT2b', tag='dps')
                nc.tensor.transpose(psT2, s2[t][:, c * 160 + 128: c * 160 + 160], id_sb)
                nc.vector.tensor_copy(s2T1[:, c * 256 + 128 * t: c * 256 + 128 * t + 128], psT2)
        s2T = [s2T0, s2T1]
        s3 = []
        for t in range(2):
            ot = dpool.tile([80, 768], mdt, name=f's3_{t}', tag=f's3_{t}')
            for (lo, hi) in ((0, 384), (384, 768)):
                ps = pdp.tile([80, 384], f32, name='dps3', tag='dps')
                mms = [k for k in range(2) if dwt['G2'][t][k] is not None]
                for i, k in enumerate(mms):
                    nc.tensor.matmul(ps, C(f'G2_{t}_{k}'), s2T[k][:, lo:hi],
                                     start=(i == 0), stop=(i == len(mms) - 1))
                nc.vector.tensor_copy(ot[:, lo:hi], ps)
            s3.append(ot)
        s3A, s3B = s3

        # ================= L3 =================
        in3 = dpool.tile([72, 3 * 128], f32, name='in3', tag='in3')
        nc.sync.dma_start(out=in3.rearrange('p (c s) -> p c s', c=3),
                          in_=asf32(s3A.rearrange('p (c s) -> p c s', c=3)[4:76, :, 0:128]))
        s4 = dpool.tile([64, 3 * 128], f32, name='s4', tag='s4')
        ps4c = pdp.tile([64, 384], f32, name='dps4', tag='dps')
        nc.tensor.matmul(ps4c, C('F3_0_0'), in3[:], start=True, stop=True)
        nc.vector.tensor_copy(s4, ps4c)
        s4T = dpool.tile([128, 3 * 64], f32, name='s4T', tag='s4T')
        for c in range(3):
            psT = pdp.tile([128, 64], f32, name='dpsT3', tag='dps')
            nc.tensor.transpose(psT, s4[:, c * 128: c * 128 + 128], C('ident')[0:64, 0:64])
            nc.vector.tensor_copy(s4T[:, c * 64: c * 64 + 64], psT)
        s5 = []
        for t in range(2):
            ot = dpool.tile([64, 192], mdt, name=f's5_{t}', tag=f's5_{t}')
            ps = pdp.tile([64, 192], f32, name='dps5', tag='dps')
            nc.tensor.matmul(ps, C(f'G3_{t}_0'), s4T[:], start=True, stop=True)
            nc.vector.tensor_copy(ot, ps)
            s5.append(ot)
        s5a, s5b = s5

        # ================= attention =================
        def cview(t_):
            return t_.rearrange('p (c s) -> p c s', c=3)
        att_s = dpool.tile([64, 96], f32, name='att_s', tag='att_s')
        nc.vector.tensor_add(cview(att_s), asf32(cview(s5a)[:, :, 32:64]), asf32(cview(s5b)[:, :, 0:32]))
        att_m = dpool.tile([64, 32], f32, name='att_m', tag='att_m')
        nc.vector.tensor_max(att_m, att_s[:, 0:32], att_s[:, 32:64])
        nc.vector.tensor_max(att_m, att_m, att_s[:, 64:96])
        att_u = dpool.tile([64, 96], f32, name='att_u', tag='att_u')
        for c in range(3):
            nc.vector.tensor_sub(att_u[:, 32 * c:32 * c + 32], att_s[:, 32 * c:32 * c + 32], att_m)
        att_e = dpool.tile([64, 96], f32, name='att_e', tag='att_e')
        nc.scalar.activation(att_e, att_u, AF.Exp)
        att_d = dpool.tile([64, 32], f32, name='att_d', tag='att_d')
        nc.vector.tensor_add(att_d, att_e[:, 0:32], att_e[:, 32:64])
        nc.vector.tensor_add(att_d, att_d, att_e[:, 64:96])
        att_r = dpool.tile([64, 32], f32, name='att_r', tag='att_r')
        nc.vector.reciprocal(att_r, att_d)
        llat = dpool.tile([64, 96], mdt, name='llat', tag='llat')
        att_t = dpool.tile([64, 96], f32, name='att_t', tag='att_t')
        for c in range(3):
            sl = slice(32 * c, 32 * c + 32)
            ll3_c = asf32(cview(s5a)[:, c, 0:32])
            nc.vector.tensor_mul(att_t[:, sl], att_e[:, sl], att_r)
            # llat_c = (att_c + 1) * ll3_c
            nc.vector.scalar_tensor_tensor(llat[:, sl], att_t[:, sl], 1.0, ll3_c,
                                           mybir.AluOpType.add, mybir.AluOpType.mult)

        # ================= MLP =================
        # staging fills: one DMA per staging row (4/tile). A K=2 absorber
        # matmul before each staging's MLP keeps every matmul at <=2 sem waits.
        INNER = {1: 128, 2: 128, 3: 32, 'yl': 32}

        def fill_staging(stg, lvl, g, jb):
            inner = INNER[lvl]
            dstv = stg.rearrange('p (a b) -> p a b', b=inner)

            def row_src(vch, r):
                if lvl == 'yl':
                    return cview(llat)[:, vch, :]
                c, bn = vch // 3, BANDS[vch % 3]
                wb, hb = BSEL[bn]
                if lvl == 1:
                    if g == 2:
                        w0 = 256 + 64 * r + 16 * jb
                    else:
                        w0 = 256 * wb + 16 * jb
                    return cview(b1[w0 // 128])[w0 % 128: w0 % 128 + 16, c,
                                                192 * hb + 32: 192 * hb + 160]
                if lvl == 2:
                    h0_ = 8 + (16 * r if g == 2 else 16 * jb)
                    src = s3B if hb else s3A
                    return cview(src)[h0_:h0_ + 16, c, 128 * wb: 128 * wb + 128]
                src = s5b if wb else s5a
                if g == 2:
                    return cview(src)[16 * r: 16 * r + 16, c, 32 * hb: 32 * hb + 32]
                return cview(src)[0:64, c, 32 * hb: 32 * hb + 32]

            vcs = GROUP_VCH[lvl][g]
            dmas = []
            for r in range(4):
                vch = vcs[r] if vcs[r] is not None else vcs[0]
                src = row_src(vch, r)
                na = src.shape[0]
                dmas.append(nc.sync.dma_start(out=dstv[r:r + 1, 0:na, :], in_=src))
            return dmas

        sctr = [0]

        def run_group(lvl, g, n_stg, store_fn, stg_px=2048):
            ntiles = stg_px // 512
            for jb in range(n_stg):
                stg = spool.tile([4, 2048], mdt, name=f'stg{sctr[0]}', tag='stg')
                sctr[0] += 1
                fill_staging(stg, lvl, g, jb)
                for kk in range(ntiles):
                    rhs = stg[:, 512 * kk: 512 * kk + 512]
                    ps1 = php.tile([128, 512], f32, name='hps1', tag='hps')
                    nc.tensor.matmul(ps1, CR(f'{lvl}_{g}_W1'), rhs, start=True, stop=True)
                    h1 = hpool.tile([128, 512], mdt, name='h1', tag='h1')
                    nc.scalar.activation(h1, ps1, AF.Tanh, bias=C(f'{lvl}_{g}_B1'))
                    ps2 = php.tile([128, 512], f32, name='hps2', tag='hps')
                    nc.tensor.matmul(ps2, CR(f'{lvl}_{g}_W2'), h1[:], start=True, stop=True)
                    h2 = hpool.tile([128, 512], mdt, name='h2', tag='h2')
                    nc.scalar.activation(h2, ps2, AF.Tanh, bias=C(f'{lvl}_{g}_B2'))
                    ps3 = php.tile([128, 512], f32, name='hps3', tag='hps')
                    nc.tensor.matmul(ps3, CR(f'{lvl}_{g}_W3'), h2[:], start=True, stop=True)
                    h3 = hpool.tile([128, 512], mdt, name='h3', tag='h3')
                    nc.scalar.activation(h3, ps3, AF.Tanh, bias=C(f'{lvl}_{g}_B3'))
                    pso = pop.tile([4, 512], f32, name='ops', tag='ops')
                    nc.tensor.matmul(pso, CR(f'{lvl}_{g}_W4'), h3[:], start=True, stop=True)
                    osb = opool.tile([4, 512], f32, name='osb', tag='osb')
                    nc.vector.tensor_scalar_add(osb, pso, C(f'{lvl}_{g}_B4'))
                    store_fn(jb, kk, osb)

        for lvl in (1, 2, 3):
            npx = LVL_NPX[lvl]
            oy = OY[lvl]
            for g in range(2):
                def store(jb, kk, osb, g=g, oy=oy):
                    o0 = jb * 2048 + kk * 512
                    return nc.sync.dma_start(out=oy[4 * g:4 * g + 4, o0:o0 + 512], in_=osb[:])
                run_group(lvl, g, npx // 2048, store)
            qn = npx // 4
            oyv = oy.rearrange('c (r n) -> c r n', r=4)

            def store(jb, kk, osb, oyv=oyv):
                o0 = jb * 2048 + kk * 512
                return nc.sync.dma_start(out=oyv[8:9, :, o0:o0 + 512], in_=osb[:])
            run_group(lvl, 2, max(1, qn // 2048), store, stg_px=min(2048, qn))

        def store_yl(jb, kk, osb):
            return nc.sync.dma_start(out=oxe[0:3, kk * 512: kk * 512 + 512], in_=osb[0:3, :])
        run_group('yl', 0, 1, store_yl)

    nc.compile()
    return nc


# ---------------------------------------------------------------------------
# host entry point
# ---------------------------------------------------------------------------

_CACHE = {}


def _in_maps(inputs, dwt, packs):
    c32, c32r = pack_consts(dwt, packs)
    x = np.asarray(inputs['x'], np.float32)
    return [{'xh': shard_input(x, core), 'c32': c32, 'c32r': c32r} for core in range(8)]


def assemble(outs_per_core):
    out_xe = np.zeros((4, 3, 64, 64), np.float32)
    y1 = np.zeros((4, 9, 256, 256), np.float32)
    y2 = np.zeros((4, 9, 128, 128), np.float32)
    y3 = np.zeros((4, 9, 64, 64), np.float32)
    s1m, s2m, s3m = np.array(SLOTS[1]), np.array(SLOTS[2]), np.array(SLOTS[3])
    for core, o in enumerate(outs_per_core):
        b, half = core // 2, core % 2
        t1 = np.empty_like(o['o_y1'])
        t1[s1m] = o['o_y1']
        t2 = np.empty_like(o['o_y2'])
        t2[s2m] = o['o_y2']
        t3 = np.empty_like(o['o_y3'])
        t3[s3m] = o['o_y3']
        y1[b, :, 128 * half:128 * half + 128, :] = t1.reshape(9, 256, 128).transpose(0, 2, 1)
        y2[b, :, 64 * half:64 * half + 64, :] = t2.reshape(9, 64, 128)
        y3[b, :, 32 * half:32 * half + 32, :] = t3.reshape(9, 64, 32).transpose(0, 2, 1)
        out_xe[b, :, 32 * half:32 * half + 32, :] = o['o_xe'].reshape(3, 64, 32).transpose(0, 2, 1)
    return (out_xe, y1, y2, y3)


def _run(inputs, **kw):
    from concourse.bass_utils import run_bass_kernel_spmd
    dwt = make_dwt_consts()
    packs = make_mlp_consts({k: np.asarray(v, np.float32) for k, v in inputs.items() if k != 'x'})
    if 'nc' not in _CACHE:
        _CACHE['nc'] = build_program(dwt)
    nc = _CACHE['nc']
    in_maps = _in_maps(inputs, dwt, packs)
    res = run_bass_kernel_spmd(nc, in_maps, list(range(8)), **kw)
    return assemble(res.results), res


def kernel(**inputs):
    return _run(inputs)[0]
